# revision 9
# baseline (speedup 1.0000x reference)
"""StyleGAN2-style modulated 3x3 conv (B=8, Ci=Co=512, H=W=32) on 8 TRN2
NeuronCores, via 1-D Winograd F(4,3) along y, fp16 compute.

Sharding: data-parallel over batch, one sample per core (embarrassingly
parallel, no collectives).

Per core the conv is decomposed with 1-D Winograd F(4,3) applied to the
ky axis: the 3 ky taps collapse into 6 transform phases a=0..5, each
y-tile producing 4 output rows — 2x fewer MACs than direct conv
(288 N=256 matmuls instead of 288 N=512):

  V_a[ty, x'] = sum_r Bt[a,r] * pad[4*ty + r, x']            (DVE fp16)
  M_a[co]     = sum_{kx,ci} U1[a,kx,ci,co] V_a[ci][:, kx:kx+32]   (PE)
  out[4ty+p]  = (sum_a At[p,a] M_a) * rs + bias

with
  Bt = [[4,0,-5,0,1,0],[0,-4,-4,1,1,0],[0,4,-4,-1,1,0],
        [0,-2,-1,2,1,0],[0,2,-1,-2,1,0],[0,4,0,-5,0,1]]
  At = [[1,1,1,1,1,0],[0,1,-1,2,-2,0],[0,1,1,4,4,0],[0,1,-1,8,-8,1]]
  U1[a,kx] = sum_ky G[a,ky] w[:,:,ky,kx]
  G  = [[1/4,0,0],[-1/6,-1/6,-1/6],[-1/6,1/6,-1/6],
        [1/24,1/12,1/6],[1/24,-1/12,1/6],[0,0,1]]

U1 is an input-independent weight transform folded on the host (same
category as the baseline's w2 fold); all input-dependent math runs on
device. fp16 (10 mantissa bits) absorbs the Winograd transforms'
amplification — simulated rel err 2.2e-3 vs 2.7e-3 for the direct
bf16 conv.

Demod as in the baseline: conv runs on raw (unscaled) weights and the
per-(b,co) norm uses w2 = sum_k w^2 with compensated eps:
  out = conv / sqrt(sum_ci ys^2 * w2 + 1e-8*Ci*K^2) + bias
"""

import numpy as np
import ml_dtypes

import concourse.mybir as mybir
from concourse import bacc
from concourse.tile import TileContext
from concourse.bass_utils import run_bass_kernel_spmd

B = 8
CI = 512
CO = 512
H = W = 32
NCI = CI // 128
NCO = CO // 128
ALPHA = 6          # F(4,3): 6 transform phases
MOUT = 4           # output rows per tile
NTY = H // MOUT    # 8 y-tiles
NSLOT = ALPHA * 3 + 1  # 18 conv slots (a*3+kx) + 1 w2 slot
PADH = 34
PADW = 36          # cols: [0..1]=left border, [2..33]=x, [34..35]=right border
EPS_EFF = 1e-8 * CI * 9

F32 = mybir.dt.float32
F16 = mybir.dt.float16
AF = mybir.ActivationFunctionType
ALU = mybir.AluOpType


def build_nc():
    nc = bacc.Bacc("TRN2", target_bir_lowering=False, debug=False)

    x_ext = nc.declare_dram_parameter("x", [NCI, 128, H, W], F16, isOutput=False)
    # cols 0..3 = y_s per ci-tile, cols 4..7 = bias per co-tile
    yb_ext = nc.declare_dram_parameter("yb", [128, 2 * NCI], F32, isOutput=False)
    # [jo, jci, ci_p, slot(18 conv + w2), co_c] fp16
    wt_ext = nc.declare_dram_parameter(
        "wt", [NCO, NCI, 128, NSLOT, 128], F16, isOutput=False
    )
    out_ext = nc.declare_dram_parameter("out", [NCO, 128, H * W], F16, isOutput=True)

    with TileContext(nc) as tc:
        with (
            tc.tile_pool(name="singles", bufs=1) as singles,
            tc.tile_pool(name="wts", bufs=1) as wts,
            tc.tile_pool(name="pads", bufs=1) as pads,
            tc.tile_pool(name="vts", bufs=1) as vts,
            tc.tile_pool(name="vtmp", bufs=2) as vtmp,
            tc.tile_pool(name="xin", bufs=4) as xin,
            tc.tile_pool(name="mbs", bufs=8) as mbs,
            tc.tile_pool(name="zts", bufs=2) as zts,
            tc.tile_pool(name="outs", bufs=2) as outs,
            tc.tile_pool(name="cps", bufs=6, space="PSUM") as cps,
            tc.tile_pool(name="dps", bufs=1, space="PSUM") as dps,
            tc.tile_pool(name="wps", bufs=1, space="PSUM") as wps,
        ):
            # ---- input DMAs ----
            xt_sb = [
                xin.tile([128, H, W], F16, tag=f"x{j}", name=f"xt{j}")
                for j in range(NCI)
            ]
            yb_sb = singles.tile([128, 2 * NCI], F32)
            nc.sync.dma_start(out=xt_sb[0], in_=x_ext[0])
            nc.sync.dma_start(out=yb_sb, in_=yb_ext[:, :])

            NW = NCO * NCI
            wt_sb = [None] * NW

            def wdma(k):
                w = wts.tile([128, NSLOT, 128], F16, tag=f"wt{k}")
                nc.gpsimd.dma_start(out=w, in_=wt_ext[k // NCI, k % NCI])
                wt_sb[k] = w

            def wt_slice(jo, jci, s):
                return wt_sb[jo * NCI + jci][:, s, :]

            wscr = singles.tile([1, 1], F16)
            xscr = singles.tile([1, 1], F16)

            # first two weight tiles in flight alongside x0
            wdma(0)
            wdma(1)

            # pad border memsets early on gpsimd (no deps; must precede
            # the throttled weight chain below or V-transforms would
            # queue behind late weight DMAs in the gpsimd FIFO)
            pad_sb = []
            for j in range(NCI):
                p = pads.tile([128, PADH, PADW], F16, tag=f"pad{j}")
                nc.gpsimd.memset(p[:, 0, :], 0.0)
                nc.gpsimd.memset(p[:, PADH - 1, :], 0.0)
                nc.gpsimd.memset(p[:, 1 : PADH - 1, 0:2], 0.0)
                nc.gpsimd.memset(p[:, 1 : PADH - 1, PADW - 2 : PADW], 0.0)
                pad_sb.append(p)

            # once x0 has landed, issue x1..x3 concurrently (they share
            # bandwidth round-robin and all land early), then run the
            # weight stream SEQUENTIALLY with <=2 transfers in flight
            # (the DMA rings round-robin across active transfers, so an
            # unthrottled burst would make the first-needed tile land
            # as late as the last one).
            nc.gpsimd.tensor_copy(out=xscr, in_=xt_sb[0][0:1, 0, 0:1])
            for j in range(1, NCI):
                nc.gpsimd.dma_start(out=xt_sb[j], in_=x_ext[j])
            for k in range(2, NW):
                nc.gpsimd.tensor_copy(out=wscr, in_=wt_sb[k - 2][0:1, 0, 0:1])
                wdma(k)

            # ---- PE warm-up: throwaway matmuls on memset data so the
            # HAM clock gate starts releasing before the real stream ----
            warm_lhs = singles.tile([128, 1], F16)
            nc.vector.memset(warm_lhs, 1.0)
            warm_rhs = singles.tile([128, 512], F16)
            nc.vector.memset(warm_rhs, 0.5)
            warm_ps = wps.tile([1, 512], F32)
            N_WARM = 8
            for i in range(N_WARM):
                nc.tensor.matmul(
                    out=warm_ps,
                    lhsT=warm_lhs,
                    rhs=warm_rhs,
                    start=(i == 0),
                    stop=(i == N_WARM - 1),
                )

            eps_sb = singles.tile([128, 1], F32)
            nc.vector.memset(eps_sb, EPS_EFF)

            # ---- modulate + F(4,3) y-transform per ci-tile (DVE fp16,
            # 2x mode: inner dim contiguous, rows 4B-aligned) ----
            v_sb = [[None] * NCI for _ in range(ALPHA)]
            for j in range(NCI):
                nc.vector.tensor_scalar(
                    out=pad_sb[j][:, 1 : H + 1, 2 : W + 2],
                    in0=xt_sb[j],
                    scalar1=yb_sb[:, j : j + 1],
                    scalar2=None,
                    op0=ALU.mult,
                )
                p = pad_sb[j]
                for a in range(ALPHA):
                    v_sb[a][j] = vts.tile(
                        [128, NTY, PADW], F16, tag=f"v{a}_{j}", name=f"v{a}_{j}"
                    )
                P = [p[:, r : r + 29 : 4, :] for r in range(ALPHA)]

                def tmp(tag):
                    return vtmp.tile([128, NTY, PADW], F16, tag=tag, name=tag)

                stt = nc.vector.scalar_tensor_tensor
                # V0 = 4*P0 - 5*P2 + P4
                t0 = tmp("t0")
                stt(out=t0, in0=P[2], scalar=-5.0, in1=P[4], op0=ALU.mult, op1=ALU.add)
                stt(out=v_sb[0][j], in0=P[0], scalar=4.0, in1=t0, op0=ALU.mult, op1=ALU.add)
                # V1 = -4*(P1+P2) + (P3+P4) ; V2 = 4*(P1-P2) + (P4-P3)
                s12 = tmp("s12")
                s34 = tmp("s34")
                nc.vector.tensor_add(s12, P[1], P[2])
                nc.vector.tensor_add(s34, P[3], P[4])
                stt(out=v_sb[1][j], in0=s12, scalar=-4.0, in1=s34, op0=ALU.mult, op1=ALU.add)
                d12 = tmp("d12")
                d43 = tmp("d43")
                nc.vector.tensor_sub(d12, P[1], P[2])
                nc.vector.tensor_sub(d43, P[4], P[3])
                stt(out=v_sb[2][j], in0=d12, scalar=4.0, in1=d43, op0=ALU.mult, op1=ALU.add)
                # V3 = 2*(P3-P1) + (P4-P2) ; V4 = -2*(P3-P1) + (P4-P2)
                d31 = tmp("d31")
                d42 = tmp("d42")
                nc.vector.tensor_sub(d31, P[3], P[1])
                nc.vector.tensor_sub(d42, P[4], P[2])
                stt(out=v_sb[3][j], in0=d31, scalar=2.0, in1=d42, op0=ALU.mult, op1=ALU.add)
                stt(out=v_sb[4][j], in0=d31, scalar=-2.0, in1=d42, op0=ALU.mult, op1=ALU.add)
                # V5 = 4*P1 - 5*P3 + P5
                t5 = tmp("t5")
                stt(out=t5, in0=P[3], scalar=-5.0, in1=P[5], op0=ALU.mult, op1=ALU.add)
                stt(out=v_sb[5][j], in0=P[1], scalar=4.0, in1=t5, op0=ALU.mult, op1=ALU.add)

            # ys^2 in fp16 for the demod matmuls
            ys2_sb = singles.tile([128, NCI], F16)
            nc.vector.tensor_mul(ys2_sb, yb_sb[:, 0:NCI], yb_sb[:, 0:NCI])

            xs2_ps = dps.tile([128, NCO], F32)
            rs_sb = singles.tile([128, NCO], F32)

            # ---- main stream ----
            for jo in range(NCO):
                mb = [None] * ALPHA
                for a in range(ALPHA):
                    ps = cps.tile([128, NTY, W], F32, tag="m")
                    idx = 0
                    for jci in range(NCI):
                        for kx in range(3):
                            nc.tensor.matmul(
                                out=ps,
                                lhsT=wt_slice(jo, jci, a * 3 + kx),
                                rhs=v_sb[a][jci][:, :, kx + 1 : kx + 1 + W],
                                start=(idx == 0),
                                stop=(idx == 11),
                            )
                            idx += 1
                    m = mbs.tile([128, NTY, W], F16, tag="mb", name="mb")
                    nc.scalar.activation(out=m, in_=ps, func=AF.Copy)
                    mb[a] = m
                # demod for this co-tile (tiny MMs; weights all present
                # by now so the PE never stalls on them)
                for jci in range(NCI):
                    nc.tensor.matmul(
                        out=xs2_ps[:, jo : jo + 1],
                        lhsT=wt_slice(jo, jci, NSLOT - 1),
                        rhs=ys2_sb[:, jci : jci + 1],
                        start=(jci == 0),
                        stop=(jci == NCI - 1),
                    )
                nc.scalar.activation(
                    out=rs_sb[:, jo : jo + 1],
                    in_=xs2_ps[:, jo : jo + 1],
                    func=AF.Sqrt,
                    bias=eps_sb,
                )
                nc.vector.reciprocal(
                    out=rs_sb[:, jo : jo + 1], in_=rs_sb[:, jo : jo + 1]
                )
                # combine (DVE fp16):
                #   p0 = M0+M1+M2+M3+M4, p1 = (M1-M2) + 2(M3-M4)
                #   p2 = (M1+M2) + 4(M3+M4), p3 = ((M1-M2)+M5) + 8(M3-M4)
                def zt(tag):
                    return zts.tile([128, NTY, W], F16, tag=tag, name=tag)

                stt = nc.vector.scalar_tensor_tensor
                sp = zt("sp")
                sm = zt("sm")
                tp = zt("tp")
                tm = zt("tm")
                nc.vector.tensor_add(sp, mb[1], mb[2])
                nc.vector.tensor_sub(sm, mb[1], mb[2])
                nc.vector.tensor_add(tp, mb[3], mb[4])
                nc.vector.tensor_sub(tm, mb[3], mb[4])
                u0 = zt("u0")
                z0 = zt("z0")
                z1 = zt("z1")
                z2 = zt("z2")
                z3 = zt("z3")
                u3 = zt("u3")
                nc.vector.tensor_add(u0, mb[0], sp)
                nc.vector.tensor_add(z0, u0, tp)
                stt(out=z1, in0=tm, scalar=2.0, in1=sm, op0=ALU.mult, op1=ALU.add)
                stt(out=z2, in0=tp, scalar=4.0, in1=sp, op0=ALU.mult, op1=ALU.add)
                nc.vector.tensor_add(u3, sm, mb[5])
                stt(out=z3, in0=tm, scalar=8.0, in1=u3, op0=ALU.mult, op1=ALU.add)
                # epilogue: out rows 4ty+p = Z_p * rs + bias
                ot = outs.tile([128, H, W], F16, tag="ot")
                for p, z in ((0, z0), (1, z1), (2, z2), (3, z3)):
                    nc.scalar.activation(
                        out=ot[:, p : p + 29 : 4, :],
                        in_=z,
                        func=AF.Identity,
                        bias=yb_sb[:, NCI + jo : NCI + jo + 1],
                        scale=rs_sb[:, jo : jo + 1],
                    )
                nc.sync.dma_start(out=out_ext[jo], in_=ot)

            # keep the warm-up matmuls live (cheap PSUM read at the end)
            warm_sink = singles.tile([1, 1], F32)
            nc.vector.tensor_copy(out=warm_sink, in_=warm_ps[0:1, 0:1])
    nc.compile()
    return nc


_NC_CACHE = None


def _get_nc():
    global _NC_CACHE
    if _NC_CACHE is None:
        _NC_CACHE = build_nc()
    return _NC_CACHE


def _prep_inputs(x, y_s, weight, bias):
    # Winograd weight transform (input-independent): U1[a,kx,ci,co] =
    # sum_ky G[a,ky] w[co,ci,ky,kx]; slot 18 = w2 = sum_k w^2.
    G = np.array(
        [
            [1 / 4, 0, 0],
            [-1 / 6, -1 / 6, -1 / 6],
            [-1 / 6, 1 / 6, -1 / 6],
            [1 / 24, 1 / 12, 1 / 6],
            [1 / 24, -1 / 12, 1 / 6],
            [0, 0, 1],
        ],
        np.float64,
    )
    w64 = weight.astype(np.float64)  # [co, ci, ky, kx]
    u1 = np.einsum("ag,oigx->axio", G, w64)  # [a, kx, ci, co]
    w2 = (w64**2).sum(axis=(2, 3)).T  # [ci, co]
    full = np.concatenate(
        [u1.reshape(ALPHA * 3, CI, CO), w2[None]], axis=0
    )  # [19, ci, co]
    wtq = np.ascontiguousarray(
        full.reshape(NSLOT, NCI, 128, NCO, 128).transpose(3, 1, 2, 0, 4)
    ).astype(np.float16)
    in_maps = []
    for b in range(B):
        yb = np.empty((128, 2 * NCI), np.float32)
        yb[:, :NCI] = y_s[b].reshape(NCI, 128).T
        yb[:, NCI:] = bias.reshape(NCO, 128).T
        in_maps.append(
            {
                "x": np.ascontiguousarray(x[b].reshape(NCI, 128, H, W)).astype(
                    np.float16
                ),
                "yb": yb,
                "wt": wtq,
            }
        )
    return in_maps


def _install_trace_support():
    """Dev-only: register the axon NTFF profiling hook + disable the
    remote artifact upload so trace=True works in this container."""
    import sys
    import types

    import concourse.bass_utils as bu

    bu.upload_artifacts = lambda tmpdir: "local://" + str(tmpdir)
    if "antenv.axon_hooks" in sys.modules:
        return
    try:
        from trn_agent_boot.trn_boot import _ntff_profile_via_ctypes

        hook = _ntff_profile_via_ctypes("/opt/axon/libaxon_pjrt.so")
    except Exception:
        return
    mod = types.ModuleType("antenv.axon_hooks")
    mod.get_axon_ntff_profile_hook = lambda: hook
    mod.set_axon_ntff_profile_hook = lambda h: None
    sys.modules["antenv.axon_hooks"] = mod


def run(x, y_s, weight, bias, trace=False, tmpdir=None):
    nc = _get_nc()
    if trace:
        _install_trace_support()
    in_maps = _prep_inputs(x, y_s, weight, bias)
    res = run_bass_kernel_spmd(
        nc, in_maps, core_ids=list(range(B)), trace=trace, tmpdir=tmpdir
    )
    out = np.stack(
        [res.results[b]["out"].reshape(CO, H, W).astype(np.float32) for b in range(B)]
    )
    return out, res


def kernel(x, y_s, weight, bias):
    out, _ = run(
        np.asarray(x, dtype=np.float32),
        np.asarray(y_s, dtype=np.float32),
        np.asarray(weight, dtype=np.float32),
        np.asarray(bias, dtype=np.float32),
    )
    return out
